# revision 1
# baseline (speedup 1.0000x reference)
# MMoE Trainium2 Bass kernel.
#
# Reference computation (per batch row x of size 1024):
#   per expert e:  h = x@W1[e]+b1[e]; g1 = gelu(LN(h)*ln_g+ln_b); eo = gelu(g1@W2[e]+b2[e])
#   gates (3 tasks): gh = gelu([x,cemb]@Gw1+Gb1); w = softmax(gh@Gw2+Gb2)
#   out[t] = sum_e w[t,e] * eo[e]
#
# Strategy: data-parallel over batch across 8 cores (2048 rows each, processed in
# 2 halves of 1024 so SBUF fits).  All matmuls run in bf16 with fp32 PSUM
# accumulation.  Expert layer 1 runs "transposed" (features on partitions) so the
# LayerNorm scale/bias fold into the Gelu activation op and layer 2 needs no
# transposes; layer 2 flips back to batch-on-partitions so the softmax gate
# weights apply as per-partition scalars.  LN mean comes from an extra
# weight column (sum of W1 columns) computed in the same matmul; the mean
# subtract is fused into the PSUM->SBUF drain (scalar_tensor_tensor); LN
# variance via a DVE tree-add of squared tiles plus one ones-vector matmul.
# Biases (all zero for this model's inputs) fold in via a ones row / extra
# weight rows prepared on the host.
import numpy as np
import ml_dtypes

_BF16 = ml_dtypes.bfloat16

B_FULL = 16384
IN_DIM = 1024
D_HID = 1024
D_EXP = 512
NE = 8
NT = 3
DC = 64
GH = 96  # 3 tasks x 32 gate hidden, concatenated
N_CORES = 8
EPS = 1e-5


def build_program(BC=2048, HALF=1024, has_b1=False, has_b2=False, has_gb2=False):
    import concourse.bass as bass
    import concourse.mybir as mybir
    from concourse import bacc
    from concourse.tile import TileContext

    dt = mybir.dt
    F32 = dt.float32
    BF = dt.bfloat16
    AF = mybir.ActivationFunctionType
    ALU = mybir.AluOpType

    NHALF = BC // HALF
    NBCOL = HALF // 512
    NBT = HALF // 128
    KI = IN_DIM // 128
    KH = D_HID // 128

    nc = bacc.Bacc("TRN2", target_bir_lowering=False)

    xt = nc.dram_tensor("xt", [IN_DIM, BC], BF, kind="ExternalInput")
    cta = nc.dram_tensor("cta", [DC + 1, BC], BF, kind="ExternalInput")
    w1f = nc.dram_tensor("w1f", [NE, IN_DIM + 1, D_HID + 1], BF, kind="ExternalInput")
    w2a = nc.dram_tensor("w2a", [NE, D_HID + 1, D_EXP], BF, kind="ExternalInput")
    g1t = nc.dram_tensor("g1t", [IN_DIM, GH], BF, kind="ExternalInput")
    g1b = nc.dram_tensor("g1b", [DC + 1, GH], BF, kind="ExternalInput")
    g2bd = nc.dram_tensor("g2bd", [GH, NT * NE], BF, kind="ExternalInput")
    g2bias = nc.dram_tensor("g2bias", [1, NT * NE], BF, kind="ExternalInput")
    lng = nc.dram_tensor("lng", [128, NE * KH], F32, kind="ExternalInput")
    lnb = nc.dram_tensor("lnb", [128, NE * KH], F32, kind="ExternalInput")
    outs = [
        nc.dram_tensor(f"out{t}", [BC, D_EXP], F32, kind="ExternalOutput")
        for t in range(NT)
    ]

    with TileContext(nc) as tc:
        with (
            tc.tile_pool(name="consts", bufs=1) as consts,
            tc.tile_pool(name="perhalf", bufs=1) as perhalf,
            tc.tile_pool(name="perhalf2", bufs=2) as perhalf2,
            tc.tile_pool(name="weights", bufs=2) as weights,
            tc.tile_pool(name="work", bufs=2) as work,
            tc.tile_pool(name="work1", bufs=1) as work1,
            tc.tile_pool(name="work3", bufs=2) as work3,
            tc.tile_pool(name="ph", bufs=4, space="PSUM") as ph_pool,
            tc.tile_pool(name="pmc", bufs=1, space="PSUM") as pmc_pool,
            tc.tile_pool(name="psq", bufs=1, space="PSUM") as psq_pool,
            tc.tile_pool(name="pz", bufs=2, space="PSUM") as pz_pool,
            tc.tile_pool(name="dscratch", bufs=2, space="DRAM") as dscratch,
        ):
            # ---- constants ----
            ones_row = consts.tile([1, HALF], BF, tag="ones_row")
            nc.vector.memset(ones_row, 1.0)
            ones_col = consts.tile([128, 1], BF, tag="ones_col")
            nc.vector.memset(ones_col, 1.0)
            eps_sb = consts.tile([1, 1], F32, tag="eps")
            nc.vector.memset(eps_sb, EPS)
            lng_sb = consts.tile([128, NE * KH], F32, tag="lng")
            nc.sync.dma_start(out=lng_sb, in_=lng[:, :])
            lnb_sb = consts.tile([128, NE * KH], F32, tag="lnb")
            nc.sync.dma_start(out=lnb_sb, in_=lnb[:, :])
            g1t_sb = consts.tile([128, KI, GH], BF, tag="g1t")
            nc.sync.dma_start(
                out=g1t_sb, in_=g1t[:, :].rearrange("(k p) m -> p k m", p=128)
            )
            g1b_sb = consts.tile([DC + 1, GH], BF, tag="g1b")
            nc.sync.dma_start(out=g1b_sb, in_=g1b[:, :])
            g2bd_sb = consts.tile([GH, NT * NE], BF, tag="g2bd")
            nc.sync.dma_start(out=g2bd_sb, in_=g2bd[:, :])
            g2bias_sb = consts.tile([1, NT * NE], BF, tag="g2bias")
            nc.sync.dma_start(out=g2bias_sb, in_=g2bias[:, :])

            for half in range(NHALF):
                hs = slice(half * HALF, (half + 1) * HALF)

                xt_sb = perhalf2.tile([128, KI, HALF], BF, tag="xt")
                xr = xt[:, hs].rearrange("(k p) b -> p k b", p=128)
                for q in range(4):
                    nc.sync.dma_start(
                        out=xt_sb[:, 2 * q : 2 * q + 2, :], in_=xr[:, 2 * q : 2 * q + 2, :]
                    )
                cta_sb = perhalf2.tile([DC + 1, HALF], BF, tag="cta")
                nc.sync.dma_start(out=cta_sb, in_=cta[:, hs])

                # ---------------- gates ----------------
                ghT_sb = perhalf2.tile([GH, HALF], BF, tag="ghT")
                for c in range(NBCOL):
                    cs = slice(c * 512, (c + 1) * 512)
                    gh_ps = ph_pool.tile([GH, 512], F32, tag="ph")
                    for k in range(KI):
                        nc.tensor.matmul(
                            gh_ps,
                            g1t_sb[:, k, :],
                            xt_sb[:, k, cs],
                            start=(k == 0),
                            stop=False,
                        )
                    nc.tensor.matmul(
                        gh_ps, g1b_sb[:, :], cta_sb[:, cs], start=False, stop=True
                    )
                    nc.scalar.activation(ghT_sb[:, cs], gh_ps, AF.Gelu)

                w_sb = perhalf2.tile([128, NBT, NT * NE], F32, tag="w")
                nmx = perhalf2.tile([128, NBT * NT], F32, tag="nmx")
                ssum = perhalf2.tile([128, NBT * NT], F32, tag="ssum")
                rs = perhalf2.tile([128, NBT * NT], F32, tag="rs")
                for bt in range(NBT):
                    bs = slice(bt * 128, (bt + 1) * 128)
                    lg_ps = pz_pool.tile([128, NT * NE], F32, tag="pz")
                    nc.tensor.matmul(
                        lg_ps,
                        ghT_sb[:, bs],
                        g2bd_sb[:, :],
                        start=True,
                        stop=not has_gb2,
                    )
                    if has_gb2:
                        nc.tensor.matmul(
                            lg_ps,
                            ones_row[0:1, 0:128],
                            g2bias_sb[:, :],
                            start=False,
                            stop=True,
                        )
                    nc.scalar.copy(w_sb[:, bt, :], lg_ps)
                # softmax over the expert axis (groups of NE in the free dim)
                nc.vector.tensor_reduce(
                    nmx[:, :],
                    w_sb[:].rearrange("p a (t e) -> p a t e", e=NE),
                    axis=mybir.AxisListType.X,
                    op=ALU.max,
                    negate=True,
                )
                for bt in range(NBT):
                    for t in range(NT):
                        j = bt * NT + t
                        nc.scalar.activation(
                            w_sb[:, bt, t * NE : (t + 1) * NE],
                            w_sb[:, bt, t * NE : (t + 1) * NE],
                            AF.Exp,
                            bias=nmx[:, j : j + 1],
                            accum_out=ssum[:, j : j + 1],
                        )
                nc.vector.reciprocal(rs[:, :], ssum[:, :])
                for bt in range(NBT):
                    for t in range(NT):
                        j = bt * NT + t
                        nc.vector.tensor_scalar_mul(
                            w_sb[:, bt, t * NE : (t + 1) * NE],
                            w_sb[:, bt, t * NE : (t + 1) * NE],
                            rs[:, j : j + 1],
                        )

                accs = [
                    perhalf.tile(
                        [128, NBT, D_EXP], F32, tag=f"acc{t}", name=f"acc{t}"
                    )
                    for t in range(NT)
                ]

                # ---------------- experts ----------------
                for e in range(NE):
                    w1_sb = weights.tile([128, KI, D_HID + 1], BF, tag="w1")
                    w1r = w1f[e, 0:IN_DIM, :].rearrange("(k p) m -> p k m", p=128)
                    for q in range(4):
                        nc.sync.dma_start(
                            out=w1_sb[:, 2 * q : 2 * q + 2, :],
                            in_=w1r[:, 2 * q : 2 * q + 2, :],
                        )
                    if has_b1:
                        w1b_sb = weights.tile([1, D_HID + 1], BF, tag="w1b")
                        nc.sync.dma_start(out=w1b_sb, in_=w1f[e, IN_DIM : IN_DIM + 1, :])
                    w2_sb = weights.tile([128, KH, D_EXP], BF, tag="w2")
                    w2r = w2a[e, 0:D_HID, :].rearrange("(k p) m -> p k m", p=128)
                    nc.sync.dma_start(
                        out=w2_sb[:, 0 : KH // 2, :], in_=w2r[:, 0 : KH // 2, :]
                    )
                    nc.sync.dma_start(
                        out=w2_sb[:, KH // 2 :, :], in_=w2r[:, KH // 2 :, :]
                    )
                    if has_b2:
                        w2b_sb = weights.tile([1, D_EXP], BF, tag="w2b")
                        nc.sync.dma_start(out=w2b_sb, in_=w2a[e, D_HID : D_HID + 1, :])

                    for c in range(NBCOL):
                        cs = slice(c * 512, (c + 1) * 512)
                        # mean column: mc = sum_hid(h) for the 512 batch cols
                        mc_ps = pmc_pool.tile([1, 512], F32, tag="pmc")
                        for k in range(KI):
                            nc.tensor.matmul(
                                mc_ps,
                                w1_sb[:, k, D_HID : D_HID + 1],
                                xt_sb[:, k, cs],
                                start=(k == 0),
                                stop=(k == KI - 1) and not has_b1,
                            )
                        if has_b1:
                            nc.tensor.matmul(
                                mc_ps,
                                w1b_sb[0:1, D_HID : D_HID + 1],
                                ones_row[0:1, cs],
                                start=False,
                                stop=True,
                            )
                        mu = work1.tile([1, 512], F32, tag="mu")
                        nc.scalar.activation(mu, mc_ps, AF.Copy, scale=1.0 / D_HID)
                        mu_d = dscratch.tile([1, 512], F32, tag="mu_d")
                        nc.sync.dma_start(out=mu_d, in_=mu[0:1, :])
                        mu_b = work1.tile([128, 512], F32, tag="mu_b")
                        nc.sync.dma_start(
                            out=mu_b, in_=mu_d[:].to_broadcast([128, 512])
                        )

                        hc_sb = work.tile([128, KH, 512], F32, tag="hc")
                        hsq = work1.tile([128, KH, 512], BF, tag="hsq")
                        for m in range(KH):
                            hp = ph_pool.tile([128, 512], F32, tag="ph")
                            for k in range(KI):
                                nc.tensor.matmul(
                                    hp,
                                    w1_sb[:, k, m * 128 : (m + 1) * 128],
                                    xt_sb[:, k, cs],
                                    start=(k == 0),
                                    stop=(k == KI - 1) and not has_b1,
                                )
                            if has_b1:
                                nc.tensor.matmul(
                                    hp,
                                    w1b_sb[0:1, m * 128 : (m + 1) * 128],
                                    ones_row[0:1, cs],
                                    start=False,
                                    stop=True,
                                )
                            # drain + mean-subtract fused: hc = hp - mu
                            nc.vector.scalar_tensor_tensor(
                                hc_sb[:, m, :], hp, 1.0, mu_b,
                                op0=ALU.mult, op1=ALU.subtract,
                            )
                            nc.scalar.activation(hsq[:, m, :], hc_sb[:, m, :], AF.Square)
                        # var*H = sum_hid(hc^2): DVE tree-add the 8 hid tiles,
                        # then one ones-vector matmul for the partition reduce
                        for step in (4, 2, 1):
                            for i in range(step):
                                nc.vector.tensor_add(
                                    hsq[:, i, :], hsq[:, i, :], hsq[:, i + step, :]
                                )
                        sq_ps = psq_pool.tile([1, 512], F32, tag="psq")
                        nc.tensor.matmul(
                            sq_ps, ones_col[:, 0:1], hsq[:, 0, :],
                            start=True, stop=True,
                        )
                        rstd = work1.tile([1, 512], F32, tag="rstd")
                        nc.scalar.activation(
                            rstd, sq_ps, AF.Sqrt, bias=eps_sb[0:1, 0:1],
                            scale=1.0 / D_HID,
                        )
                        nc.vector.reciprocal(rstd, rstd)
                        # broadcast rstd across partitions via a DRAM bounce
                        rstd_d = dscratch.tile([1, 512], F32, tag="rstd_d")
                        nc.sync.dma_start(out=rstd_d, in_=rstd[0:1, :])
                        rstd_b = work1.tile([128, 512], F32, tag="rstd_b")
                        nc.sync.dma_start(
                            out=rstd_b, in_=rstd_d[:].to_broadcast([128, 512])
                        )

                        g1T = work1.tile([128, KH, 512], BF, tag="g1T")
                        for m in range(KH):
                            tmp = work3.tile([128, 512], F32, tag="tmp")
                            nc.vector.tensor_mul(tmp, hc_sb[:, m, :], rstd_b)
                            nc.scalar.activation(
                                g1T[:, m, :],
                                tmp,
                                AF.Gelu,
                                bias=lnb_sb[:, e * KH + m : e * KH + m + 1],
                                scale=lng_sb[:, e * KH + m : e * KH + m + 1],
                            )

                        for mb in range(4):
                            bt = c * 4 + mb
                            bs = slice(mb * 128, (mb + 1) * 128)
                            z2 = pz_pool.tile([128, D_EXP], F32, tag="pz")
                            for k in range(KH):
                                nc.tensor.matmul(
                                    z2,
                                    g1T[:, k, bs],
                                    w2_sb[:, k, :],
                                    start=(k == 0),
                                    stop=(k == KH - 1) and not has_b2,
                                )
                            if has_b2:
                                nc.tensor.matmul(
                                    z2,
                                    ones_row[0:1, bt * 128 : bt * 128 + 128],
                                    w2b_sb[0:1, :],
                                    start=False,
                                    stop=True,
                                )
                            eo = work3.tile([128, D_EXP], F32, tag="eo")
                            nc.scalar.activation(eo, z2, AF.Gelu)
                            for t in range(NT):
                                wsl = w_sb[:, bt, t * NE + e : t * NE + e + 1]
                                if e == 0:
                                    nc.vector.tensor_scalar_mul(
                                        accs[t][:, bt, :], eo, wsl
                                    )
                                else:
                                    nc.vector.scalar_tensor_tensor(
                                        accs[t][:, bt, :],
                                        eo,
                                        wsl,
                                        accs[t][:, bt, :],
                                        op0=ALU.mult,
                                        op1=ALU.add,
                                    )
                                if e == NE - 1:
                                    # stream this batch tile out as soon as the
                                    # last expert's contribution lands
                                    rows = slice(
                                        half * HALF + bt * 128,
                                        half * HALF + (bt + 1) * 128,
                                    )
                                    nc.sync.dma_start(
                                        out=outs[t][rows, :],
                                        in_=accs[t][:, bt, :],
                                    )

    nc.compile()
    return nc


def _host_prep(h_val, h_aro, cluster_id, W1, b1, ln_g, ln_b, W2, b2, emb, Gw1, Gb1, Gw2, Gb2):
    f32 = np.float32
    X = np.concatenate([h_val, h_aro], axis=1).astype(f32)
    B = X.shape[0]
    XT = np.ascontiguousarray(X.T).astype(_BF16)
    cemb = np.asarray(emb, f32)[np.asarray(cluster_id).astype(np.int64)]
    cta = np.concatenate(
        [np.ascontiguousarray(cemb.T), np.ones((1, B), f32)], axis=0
    ).astype(_BF16)

    W1 = np.asarray(W1, f32)
    b1 = np.asarray(b1, f32)
    W1a = np.concatenate([W1, b1[:, None, :]], axis=1)  # [E, 1025, 1024]
    W1s = W1a.sum(axis=2, dtype=np.float64).astype(f32)  # [E, 1025]
    w1f = np.concatenate([W1a, W1s[:, :, None]], axis=2).astype(_BF16)  # [E,1025,1025]

    W2 = np.asarray(W2, f32)
    b2 = np.asarray(b2, f32)
    w2a = np.concatenate([W2, b2[:, None, :]], axis=1).astype(_BF16)  # [E, 1025, 512]

    Gw1 = np.asarray(Gw1, f32)  # [T, 1088, 32]
    Gb1 = np.asarray(Gb1, f32)  # [T, 32]
    G1 = np.concatenate([Gw1[t] for t in range(NT)], axis=1)  # [1088, 96]
    G1b_bias = np.concatenate([Gb1[t] for t in range(NT)], axis=0)[None, :]  # [1, 96]
    g1t = np.ascontiguousarray(G1[:IN_DIM]).astype(_BF16)  # [1024, 96]
    g1b = np.concatenate([G1[IN_DIM:], G1b_bias], axis=0).astype(_BF16)  # [65, 96]

    Gw2 = np.asarray(Gw2, f32)  # [T, 32, 8]
    Gb2 = np.asarray(Gb2, f32)  # [T, 8]
    g2bd = np.zeros((GH, NT * NE), f32)
    for t in range(NT):
        g2bd[t * 32 : (t + 1) * 32, t * NE : (t + 1) * NE] = Gw2[t]
    g2bd = g2bd.astype(_BF16)
    g2bias = np.concatenate([Gb2[t] for t in range(NT)], axis=0)[None, :].astype(_BF16)

    ln_g = np.asarray(ln_g, f32)
    ln_b = np.asarray(ln_b, f32)
    KH = D_HID // 128
    lng = np.ascontiguousarray(
        ln_g.reshape(NE, KH, 128).transpose(2, 0, 1).reshape(128, NE * KH)
    ).astype(f32)
    lnb = np.ascontiguousarray(
        ln_b.reshape(NE, KH, 128).transpose(2, 0, 1).reshape(128, NE * KH)
    ).astype(f32)

    shared = dict(
        w1f=w1f, w2a=w2a, g1t=g1t, g1b=g1b, g2bd=g2bd, g2bias=g2bias,
        lng=lng, lnb=lnb,
    )
    flags = dict(
        has_b1=bool(np.any(b1)), has_b2=bool(np.any(b2)), has_gb2=bool(np.any(Gb2)),
    )
    return XT, cta, shared, flags


def kernel_run(inputs, trace=False):
    import sys
    if "/opt/trn_rl_repo" not in sys.path:
        sys.path.insert(0, "/opt/trn_rl_repo")
    from concourse.bass_utils import run_bass_kernel_spmd

    XT, cta, shared, flags = _host_prep(**inputs)
    B = XT.shape[1]
    BC = B // N_CORES

    nc = build_program(BC=BC, HALF=1024, **flags)

    in_maps = []
    for c in range(N_CORES):
        cs = slice(c * BC, (c + 1) * BC)
        m = dict(shared)
        m["xt"] = np.ascontiguousarray(XT[:, cs])
        m["cta"] = np.ascontiguousarray(cta[:, cs])
        in_maps.append(m)

    res = run_bass_kernel_spmd(
        nc, in_maps, core_ids=list(range(N_CORES)), trace=trace
    )
    outs = []
    for t in range(NT):
        outs.append(
            np.concatenate([res.results[c][f"out{t}"] for c in range(N_CORES)], axis=0)
        )
    return tuple(outs), res


def kernel(h_val, h_aro, cluster_id, W1, b1, ln_g, ln_b, W2, b2, emb, Gw1, Gb1, Gw2, Gb2):
    outs, _ = kernel_run(
        dict(
            h_val=h_val, h_aro=h_aro, cluster_id=cluster_id, W1=W1, b1=b1,
            ln_g=ln_g, ln_b=ln_b, W2=W2, b2=b2, emb=emb,
            Gw1=Gw1, Gb1=Gb1, Gw2=Gw2, Gb2=Gb2,
        )
    )
    return outs


if __name__ == "__main__":
    rng = np.random.default_rng(0)
    print("kernel module loaded")



# revision 53
# speedup vs baseline: 1.1749x; 1.1749x over previous
# MMoE Trainium2 Bass kernel — fp8 double-pumped, wide-moving edition.
#
# Reference computation (per batch row x of size 1024):
#   per expert e:  h = x@W1[e]+b1[e]; g1 = gelu(LN(h)*ln_g+ln_b); eo = gelu(g1@W2[e]+b2[e])
#   gates (3 tasks): gh = gelu([x,cemb]@Gw1+Gb1); w = softmax(gh@Gw2+Gb2)
#   out[t] = sum_e w[t,e] * eo[e]
#
# Strategy: data-parallel over batch across 8 cores (2048 rows each, in 2
# halves of 1024 columns).  Expert and gate matmuls run in fp8-e4m3 with
# DoubleRow perf mode (two 128-deep k-tiles per instruction) using an
# error-compensated 3-pass scheme: operands are split on the host into fp8
# hi + fp8 lo residual parts and the product is hi@hi + lo@hi + hi@lo (the
# lo@lo term is second-order in the quantization noise and dropped).  L1
# streams the full 1024-column half per instruction (out [128,1024] over two
# PSUM banks) to amortize per-instruction overhead.
#
# LayerNorm: W1's columns are centered on the host (the LN mean-subtract is
# linear: h - mean(h) = x @ (W1 - colmean(W1))), removing the runtime mean
# computation.  Variance: Square activation straight from PSUM (same
# activation table as Gelu) into an fp8 tile, a DVE tree-add (final level
# bf16), one ones-column matmul per 512 columns for the partition reduce,
# Sqrt + DVE reciprocal for rstd, GpSimd partition_broadcast (no DRAM
# bounce), and a DVE in-place multiply.
#
# The expert blocks (e, half) are software-pipelined one deep: PE order is
# L1(b), L2(b-1), sqsum(b), so layer-2 matmuls of the previous block fill
# the window where block b's LayerNorm chain produces g1(b), and the LN
# chain of block b runs on ACT/DVE/Pool during L1(b+1).
import numpy as np
import ml_dtypes

_BF16 = ml_dtypes.bfloat16
_E4M3 = ml_dtypes.float8_e4m3

B_FULL = 16384
IN_DIM = 1024
D_HID = 1024
D_EXP = 512
NE = 8
NT = 3
DC = 64
GH = 96  # 3 tasks x 32 gate hidden, concatenated
N_CORES = 8
EPS = 1e-5
S1 = 16.0  # host scale on W1 / gate-W1 before fp8 quantization
S2 = 16.0  # host scale on W2


def build_program(BC=2048, HALF=1024, has_b1=False, has_b2=False, has_gb2=False,
                  ln_affine=False):
    import concourse.bass as bass
    import concourse.bass_isa as bass_isa
    import concourse.mybir as mybir
    from concourse import bacc
    from concourse import library_config
    from concourse.tile import TileContext

    dt = mybir.dt
    F32 = dt.float32
    BF = dt.bfloat16
    F8 = dt.float8e4
    AF = mybir.ActivationFunctionType
    ALU = mybir.AluOpType
    DR = mybir.MatmulPerfMode.DoubleRow

    NHALF = BC // HALF
    NBT = HALF // 128
    KI = IN_DIM // 128
    KH = D_HID // 128
    KP = KI // 2  # k-tile pairs per full contraction

    nc = bacc.Bacc("TRN2", target_bir_lowering=False)

    xh = nc.dram_tensor("xh", [IN_DIM, BC], F8, kind="ExternalInput")
    xl = nc.dram_tensor("xl", [IN_DIM, BC], F8, kind="ExternalInput")
    cta = nc.dram_tensor("cta", [DC + 1, BC], BF, kind="ExternalInput")
    w1h = nc.dram_tensor("w1h", [NE, IN_DIM, D_HID], F8, kind="ExternalInput")
    w1l = nc.dram_tensor("w1l", [NE, IN_DIM, D_HID], F8, kind="ExternalInput")
    w2f = nc.dram_tensor("w2f", [NE, D_HID, D_EXP], BF, kind="ExternalInput")
    g1th = nc.dram_tensor("g1th", [IN_DIM, GH], F8, kind="ExternalInput")
    g1tl = nc.dram_tensor("g1tl", [IN_DIM, GH], F8, kind="ExternalInput")
    g1b = nc.dram_tensor("g1b", [DC + 1, GH], BF, kind="ExternalInput")
    g2bd = nc.dram_tensor("g2bd", [GH, NT * NE], BF, kind="ExternalInput")
    g2bias = nc.dram_tensor("g2bias", [1, NT * NE], BF, kind="ExternalInput")
    if ln_affine:
        lng = nc.dram_tensor("lng", [128, NE * KH], F32, kind="ExternalInput")
        lnb = nc.dram_tensor("lnb", [128, NE * KH], F32, kind="ExternalInput")
    if has_b1:
        w1bias = nc.dram_tensor("w1bias", [NE, 1, D_HID], BF, kind="ExternalInput")
    if has_b2:
        w2bias = nc.dram_tensor("w2bias", [NE, 1, D_EXP], BF, kind="ExternalInput")
    outs = [
        nc.dram_tensor(f"out{t}", [BC, D_EXP], F32, kind="ExternalOutput")
        for t in range(NT)
    ]

    with TileContext(nc) as tc:
        with (
            tc.tile_pool(name="consts", bufs=1) as consts,
            tc.tile_pool(name="perhalf1", bufs=1) as perhalf1,
            tc.tile_pool(name="perhalf", bufs=1) as perhalf,
            tc.tile_pool(name="perhalf2", bufs=2) as perhalf2,
            tc.tile_pool(name="weights", bufs=2) as weights,
            tc.tile_pool(name="weights2", bufs=3) as weights2,
            tc.tile_pool(name="hcp", bufs=3) as hcp,
            tc.tile_pool(name="hsqp", bufs=1) as hsqp,
            tc.tile_pool(name="work3", bufs=2) as work3,
            tc.tile_pool(name="rsp", bufs=1) as rsp,
            tc.tile_pool(name="ph", bufs=3, space="PSUM") as ph_pool,
            tc.tile_pool(name="pz", bufs=2, space="PSUM") as pz_pool,
        ):
            nc.gpsimd.load_library(library_config.attn)

            # ---- constants ----
            if has_b1 or has_b2 or has_gb2:
                ones_row = consts.tile([1, 128], BF, tag="ones_row")
                nc.vector.memset(ones_row, 1.0)
            eps128 = consts.tile([128, 1], F32, tag="eps128")
            nc.vector.memset(eps128, EPS)
            if ln_affine:
                lng_sb = consts.tile([128, NE * KH], F32, tag="lng")
                nc.sync.dma_start(out=lng_sb, in_=lng[:, :])
                lnb_sb = consts.tile([128, NE * KH], F32, tag="lnb")
                nc.sync.dma_start(out=lnb_sb, in_=lnb[:, :])
            g1th_sb = consts.tile([128, KI, GH], F8, tag="g1th")
            nc.sync.dma_start(
                out=g1th_sb, in_=g1th[:, :].rearrange("(k p) m -> p k m", p=128)
            )
            g1tl_sb = consts.tile([128, KI, GH], F8, tag="g1tl")
            nc.sync.dma_start(
                out=g1tl_sb, in_=g1tl[:, :].rearrange("(k p) m -> p k m", p=128)
            )
            g1b_sb = consts.tile([DC + 1, GH], BF, tag="g1b")
            nc.sync.dma_start(out=g1b_sb, in_=g1b[:, :])
            g2bd_sb = consts.tile([GH, NT * NE], BF, tag="g2bd")
            nc.sync.dma_start(out=g2bd_sb, in_=g2bd[:, :])
            g2bias_sb = consts.tile([1, NT * NE], BF, tag="g2bias")
            nc.sync.dma_start(out=g2bias_sb, in_=g2bias[:, :])

            tail_pend = None  # block awaiting its LN tail (lag 1)
            l2_pend = None    # block awaiting its layer 2 (lag 2)

            def emit_ln_tail(st):
                # hn (DVE, in place) -> gelu (ACT, in place): hc becomes g1
                # in bf16, consumed directly as L2's stationary operand
                e, hc, rstd_b = st["e"], st["hc"], st["rstd_b"]
                for mp in range(KH // 2):
                    ms = slice(2 * mp, 2 * mp + 2)
                    for mi in range(2):
                        m = 2 * mp + mi
                        nc.vector.tensor_mul(hc[:, m, :], hc[:, m, :], rstd_b)
                    if ln_affine:
                        for mi in range(2):
                            m = 2 * mp + mi
                            col = e * KH + m
                            nc.scalar.activation(
                                hc[:, m, :], hc[:, m, :], AF.Gelu,
                                bias=lnb_sb[:, col : col + 1],
                                scale=lng_sb[:, col : col + 1],
                            )
                    else:
                        nc.scalar.activation(
                            hc[:, ms, :], hc[:, ms, :], AF.Gelu, scale=1.0 / S1
                        )

            def emit_l2(st):
                e, half, hc = st["e"], st["half"], st["hc"]
                w2f_sb = st["w2f_sb"]
                w2b_sb, w_sb, accs = st["w2b_sb"], st["w_sb"], st["accs"]
                for bt in range(NBT):
                    bs = slice(bt * 128, (bt + 1) * 128)
                    z2 = pz_pool.tile([128, D_EXP], F32, tag="pz")
                    for k in range(KH):
                        nc.tensor.matmul(
                            z2, hc[:, k, bs], w2f_sb[:, k, :],
                            start=(k == 0),
                            stop=(k == KH - 1) and not has_b2,
                        )
                    if has_b2:
                        nc.tensor.matmul(
                            z2, ones_row[0:1, 0:128], w2b_sb[0:1, :],
                            start=False, stop=True,
                        )
                    eo = work3.tile([128, D_EXP], BF, tag="eo")
                    nc.scalar.activation(eo, z2, AF.Gelu)
                    for t in range(NT):
                        wsl = w_sb[:, bt, t * NE + e : t * NE + e + 1]
                        if e == 0:
                            nc.vector.tensor_scalar_mul(accs[t][:, bt, :], eo, wsl)
                        else:
                            nc.vector.scalar_tensor_tensor(
                                accs[t][:, bt, :], eo, wsl, accs[t][:, bt, :],
                                op0=ALU.mult, op1=ALU.add,
                            )
                        if e == NE - 1:
                            rows = slice(
                                half * HALF + bt * 128, half * HALF + (bt + 1) * 128
                            )
                            # gpsimd-issued DMA: keeps the SP queue free for
                            # weight loads (these wait on the mixture STTs)
                            nc.gpsimd.dma_start(
                                out=outs[t][rows, :], in_=accs[t][:, bt, :]
                            )

            for half in range(NHALF):
                hs = slice(half * HALF, (half + 1) * HALF)

                xh_sb = perhalf2.tile([128, KI, HALF], F8, tag="xh")
                xhr = xh[:, hs].rearrange("(k p) b -> p k b", p=128)
                xl_sb = perhalf2.tile([128, KI, HALF], F8, tag="xl")
                xlr = xl[:, hs].rearrange("(k p) b -> p k b", p=128)
                for q in range(4):
                    nc.sync.dma_start(
                        out=xh_sb[:, 2 * q : 2 * q + 2, :],
                        in_=xhr[:, 2 * q : 2 * q + 2, :],
                    )
                    nc.sync.dma_start(
                        out=xl_sb[:, 2 * q : 2 * q + 2, :],
                        in_=xlr[:, 2 * q : 2 * q + 2, :],
                    )
                cta_sb = perhalf1.tile([DC + 1, HALF], BF, tag="cta")
                nc.sync.dma_start(out=cta_sb, in_=cta[:, hs])

                # ---------------- gates (emitted inside the e==0 iteration:
                # gh matmuls right after L1(e0), softmax after L2(b-2), so
                # the PE never queues behind the softmax ping-pong) ----------
                ghT_sb = perhalf1.tile([GH, HALF], BF, tag="ghT")
                w_sb = perhalf2.tile([128, NBT, NT * NE], F32, tag="w")

                def emit_gates_mm():
                    for c in range(2):
                        cs = slice(c * 512, (c + 1) * 512)
                        gh_ps = pz_pool.tile([GH, 512], F32, tag="pz")
                        for j in range(KP):
                            nc.tensor.matmul(
                                gh_ps, g1th_sb[:, 2 * j : 2 * j + 2, :],
                                xh_sb[:, 2 * j : 2 * j + 2, cs],
                                start=(j == 0), stop=False, perf_mode=DR,
                            )
                        for j in range(KP):
                            nc.tensor.matmul(
                                gh_ps, g1tl_sb[:, 2 * j : 2 * j + 2, :],
                                xh_sb[:, 2 * j : 2 * j + 2, cs],
                                start=False, stop=False, perf_mode=DR,
                            )
                        for j in range(KP):
                            nc.tensor.matmul(
                                gh_ps, g1th_sb[:, 2 * j : 2 * j + 2, :],
                                xl_sb[:, 2 * j : 2 * j + 2, cs],
                                start=False, stop=False, perf_mode=DR,
                            )
                        nc.tensor.matmul(
                            gh_ps, g1b_sb[:, :], cta_sb[:, cs], start=False,
                            stop=True,
                        )
                        nc.scalar.activation(
                            ghT_sb[:, cs], gh_ps, AF.Gelu, scale=1.0 / S1
                        )

                def emit_gates_logits():
                    for bt in range(NBT):
                        bs = slice(bt * 128, (bt + 1) * 128)
                        lg_ps = pz_pool.tile([128, NT * NE], F32, tag="pz")
                        nc.tensor.matmul(
                            lg_ps, ghT_sb[:, bs], g2bd_sb[:, :],
                            start=True, stop=not has_gb2,
                        )
                        if has_gb2:
                            nc.tensor.matmul(
                                lg_ps, ones_row[0:1, 0:128], g2bias_sb[:, :],
                                start=False, stop=True,
                            )
                        nc.scalar.copy(w_sb[:, bt, :], lg_ps)

                def emit_gates_softmax():
                    nmx = perhalf1.tile([128, NBT * NT], F32, tag="nmx")
                    ssum = perhalf1.tile([128, NBT * NT], F32, tag="ssum")
                    rs = perhalf1.tile([128, NBT * NT], F32, tag="rs")
                    nc.vector.tensor_reduce(
                        nmx[:, :],
                        w_sb[:].rearrange("p a (t e) -> p a t e", e=NE),
                        axis=mybir.AxisListType.X,
                        op=ALU.max,
                        negate=True,
                    )
                    for bt in range(NBT):
                        for t in range(NT):
                            j = bt * NT + t
                            nc.scalar.activation(
                                w_sb[:, bt, t * NE : (t + 1) * NE],
                                w_sb[:, bt, t * NE : (t + 1) * NE],
                                AF.Exp,
                                bias=nmx[:, j : j + 1],
                                accum_out=ssum[:, j : j + 1],
                            )
                    nc.vector.reciprocal(rs[:, :], ssum[:, :])
                    for bt in range(NBT):
                        for t in range(NT):
                            j = bt * NT + t
                            nc.vector.tensor_scalar_mul(
                                w_sb[:, bt, t * NE : (t + 1) * NE],
                                w_sb[:, bt, t * NE : (t + 1) * NE],
                                rs[:, j : j + 1],
                            )

                accs = [
                    perhalf.tile([128, NBT, D_EXP], F32, tag=f"acc{t}", name=f"acc{t}")
                    for t in range(NT)
                ]

                # ------------- experts (1-deep software pipeline) -------------
                for e in range(NE):
                    w1h_sb = weights.tile([128, KI, D_HID], F8, tag="w1h")
                    w1hr = w1h[e].rearrange("(k p) m -> p k m", p=128)
                    w1l_sb = weights.tile([128, KI, D_HID], F8, tag="w1l")
                    w1lr = w1l[e].rearrange("(k p) m -> p k m", p=128)
                    for q in range(2):
                        nc.sync.dma_start(
                            out=w1h_sb[:, 4 * q : 4 * q + 4, :],
                            in_=w1hr[:, 4 * q : 4 * q + 4, :],
                        )
                        nc.sync.dma_start(
                            out=w1l_sb[:, 4 * q : 4 * q + 4, :],
                            in_=w1lr[:, 4 * q : 4 * q + 4, :],
                        )
                    if has_b1:
                        w1b_sb = weights.tile([1, D_HID], BF, tag="w1b")
                        nc.sync.dma_start(out=w1b_sb, in_=w1bias[e, :, :])
                    else:
                        w1b_sb = None
                    w2f_sb = weights2.tile([128, KH, D_EXP], BF, tag="w2f")
                    w2fr = w2f[e].rearrange("(k p) m -> p k m", p=128)
                    nc.sync.dma_start(
                        out=w2f_sb[:, 0 : KH // 2, :], in_=w2fr[:, 0 : KH // 2, :]
                    )
                    nc.sync.dma_start(
                        out=w2f_sb[:, KH // 2 :, :], in_=w2fr[:, KH // 2 :, :]
                    )
                    if has_b2:
                        w2b_sb = weights2.tile([1, D_EXP], BF, tag="w2b")
                        nc.sync.dma_start(out=w2b_sb, in_=w2bias[e, :, :])
                    else:
                        w2b_sb = None

                    # ---- L1: 8 m-tiles, each [128,1024] out over 2 PSUM banks ----
                    hc = hcp.tile([128, KH, HALF], BF, tag="hc")
                    hsq = hsqp.tile([128, KH, HALF], F8, tag="hsq")
                    for m in range(KH):
                        # [128, 1024] PSUM tile (2 banks); the DR ISA caps the
                        # moving stream at 2x512, so issue per-512-col matmuls
                        hp = ph_pool.tile([128, HALF], F32, tag="ph")
                        mc = slice(m * 128, (m + 1) * 128)
                        for c in range(2):
                            cs = slice(c * 512, (c + 1) * 512)
                            hpc = hp[:, cs]
                            for j in range(KP):
                                nc.tensor.matmul(
                                    hpc, w1h_sb[:, 2 * j : 2 * j + 2, mc],
                                    xh_sb[:, 2 * j : 2 * j + 2, cs],
                                    start=(j == 0), stop=False, perf_mode=DR,
                                )
                            for j in range(KP):
                                nc.tensor.matmul(
                                    hpc, w1l_sb[:, 2 * j : 2 * j + 2, mc],
                                    xh_sb[:, 2 * j : 2 * j + 2, cs],
                                    start=False, stop=False, perf_mode=DR,
                                )
                            for j in range(KP):
                                nc.tensor.matmul(
                                    hpc, w1h_sb[:, 2 * j : 2 * j + 2, mc],
                                    xl_sb[:, 2 * j : 2 * j + 2, cs],
                                    start=False,
                                    stop=(j == KP - 1) and not has_b1,
                                    perf_mode=DR,
                                )
                            if has_b1:
                                for r in range(4):
                                    nc.tensor.matmul(
                                        hpc[:, r * 128 : (r + 1) * 128],
                                        w1b_sb[0:1, mc], ones_row[0:1, :],
                                        start=False, stop=(r == 3),
                                    )
                        # drain f32 PSUM -> bf16 SBUF, split across ACT / DVE
                        if m % 2 == 0:
                            nc.scalar.activation(hc[:, m, :], hp, AF.Copy)
                        else:
                            nc.vector.tensor_scalar_mul(hc[:, m, :], hp, 1.0)
                        # square straight from PSUM on ACT (Square shares the
                        # Gelu table); scale 1/(4*S1) puts h^2/16 in fp8, so
                        # even the final tree level's sum-of-8 stays in range
                        nc.scalar.activation(
                            hsq[:, m, :], hp, AF.Square, scale=1.0 / (4.0 * S1)
                        )
                    # Emission order sets each in-order engine queue:
                    #   PE:  L1(b) -> L2(b-2)           (no other PE deps)
                    #   ACT: drains/sq(b) -> eo(b-2) -> gelu(b-1) -> sqrt(b)
                    #   DVE: drains(b) -> mix(b-2) -> hn(b-1) -> tree(b)
                    # eo must chase the z2 banks closely (pz rotation paces
                    # L2); everything else has a full window of slack since
                    # L2 lags two blocks and rstd(b) is only needed by the
                    # b tail one window later.
                    if e == 0:
                        emit_gates_mm()
                    if l2_pend is not None:
                        emit_l2(l2_pend)
                    l2_pend = tail_pend
                    if e == 0:
                        emit_gates_logits()
                        emit_gates_softmax()
                    if tail_pend is not None:
                        emit_ln_tail(tail_pend)

                    # ---- variance tree (DVE, fp8 in place) ----
                    nc.vector.tensor_add(
                        hsq[:, 0:4, :], hsq[:, 0:4, :], hsq[:, 4:8, :]
                    )
                    nc.vector.tensor_add(
                        hsq[:, 0:2, :], hsq[:, 0:2, :], hsq[:, 2:4, :]
                    )
                    nc.vector.tensor_add(hsq[:, 0, :], hsq[:, 0, :], hsq[:, 1, :])

                    # ---- partition all-reduce (GpSimd: sums across the 128
                    # partitions and broadcasts in one op, no PSUM), then
                    # rstd = 1/sqrt(16/1024 * sum(h^2/16) + eps) ----
                    var_b = rsp.tile([128, HALF], BF, tag="var_b")
                    nc.gpsimd.partition_all_reduce(
                        var_b, hsq[:, 0, :], 128, bass_isa.ReduceOp.add
                    )
                    sig_b = rsp.tile([128, HALF], BF, tag="sig_b")
                    nc.scalar.activation(
                        sig_b, var_b, AF.Sqrt,
                        bias=eps128[:, 0:1], scale=16.0 / D_HID,
                    )
                    rstd_b = rsp.tile([128, HALF], BF, tag="rstd_b")
                    with nc.allow_low_precision(reason="bf16 rstd is ample"):
                        nc.vector.reciprocal(rstd_b, sig_b)

                    tail_pend = dict(
                        e=e, half=half, hc=hc, rstd_b=rstd_b,
                        w2f_sb=w2f_sb, w2b_sb=w2b_sb,
                        w_sb=w_sb, accs=accs,
                    )

            emit_ln_tail(tail_pend)
            emit_l2(l2_pend)
            emit_l2(tail_pend)

    nc.compile()
    return nc


def _hilo(a, s):
    a32 = np.asarray(a, np.float32) * np.float32(s)
    hi = np.clip(a32, -240.0, 240.0).astype(_E4M3)
    lo = np.clip(a32 - hi.astype(np.float32), -240.0, 240.0).astype(_E4M3)
    return hi, lo


def _host_prep(h_val, h_aro, cluster_id, W1, b1, ln_g, ln_b, W2, b2, emb, Gw1, Gb1, Gw2, Gb2):
    f32 = np.float32
    X = np.concatenate([h_val, h_aro], axis=1).astype(f32)
    B = X.shape[0]
    XT = np.ascontiguousarray(X.T)
    Xh, Xl = _hilo(XT, 1.0)
    cemb = np.asarray(emb, f32)[np.asarray(cluster_id).astype(np.int64)]
    cta = np.concatenate(
        [np.ascontiguousarray(cemb.T), np.ones((1, B), f32)], axis=0
    ).astype(_BF16)

    W1 = np.asarray(W1, f32)
    b1 = np.asarray(b1, f32)
    ln_g = np.asarray(ln_g, f32)
    ln_b = np.asarray(ln_b, f32)
    W1c = W1 - W1.mean(axis=2, keepdims=True, dtype=np.float64).astype(f32)
    w1h, w1l = _hilo(W1c, S1)
    b1c = b1 - b1.mean(axis=1, keepdims=True)
    w1bias = (S1 * b1c[:, None, :]).astype(_BF16)

    W2 = np.asarray(W2, f32)
    b2 = np.asarray(b2, f32)
    w2fd = W2.astype(_BF16)
    w2bias = b2[:, None, :].astype(_BF16)

    Gw1 = np.asarray(Gw1, f32)  # [T, 1088, 32]
    Gb1 = np.asarray(Gb1, f32)  # [T, 32]
    G1 = np.concatenate([Gw1[t] for t in range(NT)], axis=1)  # [1088, 96]
    G1b_bias = np.concatenate([Gb1[t] for t in range(NT)], axis=0)[None, :]
    g1th, g1tl = _hilo(np.ascontiguousarray(G1[:IN_DIM]), S1)
    g1b = (S1 * np.concatenate([G1[IN_DIM:], G1b_bias], axis=0)).astype(_BF16)

    Gw2 = np.asarray(Gw2, f32)  # [T, 32, 8]
    Gb2 = np.asarray(Gb2, f32)  # [T, 8]
    g2bd = np.zeros((GH, NT * NE), f32)
    for t in range(NT):
        g2bd[t * 32 : (t + 1) * 32, t * NE : (t + 1) * NE] = Gw2[t]
    g2bd = g2bd.astype(_BF16)
    g2bias = np.concatenate([Gb2[t] for t in range(NT)], axis=0)[None, :].astype(_BF16)

    # identity affine (ln_g==1, ln_b==0) folds into the wide-gelu constant
    # 1/S1 scale; anything else uses the per-partition scale/bias path
    ln_affine = bool(np.any(ln_g != 1.0) or np.any(ln_b))
    KHl = D_HID // 128
    lng = np.ascontiguousarray(
        (ln_g / S1).reshape(NE, KHl, 128).transpose(2, 0, 1).reshape(128, NE * KHl)
    ).astype(f32)
    lnb = np.ascontiguousarray(
        ln_b.reshape(NE, KHl, 128).transpose(2, 0, 1).reshape(128, NE * KHl)
    ).astype(f32)

    shared = dict(
        w1h=w1h, w1l=w1l, w2f=w2fd, g1th=g1th, g1tl=g1tl, g1b=g1b,
        g2bd=g2bd, g2bias=g2bias,
    )
    flags = dict(
        has_b1=bool(np.any(b1)), has_b2=bool(np.any(b2)),
        has_gb2=bool(np.any(Gb2)), ln_affine=ln_affine,
    )
    if flags["has_b1"]:
        shared["w1bias"] = w1bias
    if flags["has_b2"]:
        shared["w2bias"] = w2bias
    if ln_affine:
        shared["lng"] = lng
        shared["lnb"] = lnb
    return Xh, Xl, cta, shared, flags


def kernel_run(inputs, trace=False):
    import sys
    if "/opt/trn_rl_repo" not in sys.path:
        sys.path.insert(0, "/opt/trn_rl_repo")
    from concourse.bass_utils import run_bass_kernel_spmd

    Xh, Xl, cta, shared, flags = _host_prep(**inputs)
    B = Xh.shape[1]
    BC = B // N_CORES

    nc = build_program(BC=BC, HALF=1024, **flags)

    in_maps = []
    for c in range(N_CORES):
        cs = slice(c * BC, (c + 1) * BC)
        m = dict(shared)
        m["xh"] = np.ascontiguousarray(Xh[:, cs])
        m["xl"] = np.ascontiguousarray(Xl[:, cs])
        m["cta"] = np.ascontiguousarray(cta[:, cs])
        in_maps.append(m)

    res = run_bass_kernel_spmd(
        nc, in_maps, core_ids=list(range(N_CORES)), trace=trace
    )
    outs = []
    for t in range(NT):
        outs.append(
            np.concatenate([res.results[c][f"out{t}"] for c in range(N_CORES)], axis=0)
        )
    return tuple(outs), res


def kernel(h_val, h_aro, cluster_id, W1, b1, ln_g, ln_b, W2, b2, emb, Gw1, Gb1, Gw2, Gb2):
    outs, _ = kernel_run(
        dict(
            h_val=h_val, h_aro=h_aro, cluster_id=cluster_id, W1=W1, b1=b1,
            ln_g=ln_g, ln_b=ln_b, W2=W2, b2=b2, emb=emb,
            Gw1=Gw1, Gb1=Gb1, Gw2=Gw2, Gb2=Gb2,
        )
    )
    return outs


if __name__ == "__main__":
    print("kernel module loaded")
